# revision 37
# baseline (speedup 1.0000x reference)
"""Dense causal MHA (B=2, S=2048, H=16, D=128, hidden=2048) on 8 Trainium2 cores.

Sharding: data-parallel over batch (2) x tensor-parallel over head groups
(4 heads/core).  Core c handles batch c//4, heads 4*(c%4) .. 4*(c%4)+3.
Each core computes a partial output (its heads' contribution to the out
projection, with bo/4 folded in); the host sums the 4 partials per batch.

v4 layout (everything bf16 except PSUM accumulation, reciprocals and the
final output, which stay f32):
  - single pass over x: phase 1 computes K^T (roped), V and Q^T (roped) for
    the whole sequence; K/Q and V ping-pong between the two 4-bank PSUM
    pools across chunks so no segment waits on Act evictions.
  - weights and x live in DRAM in partition-major layout ([128, blocks*W])
    so whole tensors stream as a few large DMAs (small per-tile DMAs are
    dispatch-bound at ~113GB/s/queue); the startup-critical wk and x0 go
    in progressively larger pieces on two dedicated queues.
  - phase 2 is attention + out-projection only, software-pipelined with a
    3-tile lookahead so the PE never waits on the exp chain.
  - softmax denominator: three M=32 ones-matmul chains col-tiled at
    partitions 0/32/64 of one bank (groups of 3 consecutive emissions run
    concurrently on disjoint PE column groups, ~2x cheaper than M=1
    chains); a [128,128] (1/32)-matmul then sums the chain partials and
    broadcasts the denominator across partitions in one shot, deferred one
    head so the PE never waits on the DVE cast feeding it.
  - out-projection deferred one chunk so its matmuls fill the PE while the
    next chunk's attention warms up; the last chunk's stores split across
    both HWDGE queues.
"""

import sys

sys.path.insert(0, "/opt/trn_rl_repo")

from contextlib import ExitStack

import numpy as np
import ml_dtypes

import concourse.tile as tile
from concourse import bacc, mybir
from concourse.bass_utils import run_bass_kernel_spmd

S = 2048
HID = 2048
D = 128
LH = 4            # heads per core
DL = LH * D       # 512 local inner dims
SC = 512          # chunk size (q and kv)
NSC = S // SC     # 4
HCH = HID // 128  # 16 contraction chunks
N_CORES = 8

f32 = mybir.dt.float32
f32r = mybir.dt.float32r
bf16 = mybir.dt.bfloat16
Exp = mybir.ActivationFunctionType.Exp
Ident = mybir.ActivationFunctionType.Identity

_CACHE = {}


def _build_nc():
    nc = bacc.Bacc("TRN2", target_bir_lowering=False, debug=False,
                   num_devices=N_CORES)

    def din(name, shape, dt=bf16):
        return nc.dram_tensor(name, shape, dt, kind="ExternalInput").ap()

    # partition-major layouts: element [p, blk*W + c] holds the logical
    # [128*blk + p, c] entry, so each 128-row tile block is a contiguous
    # free-dim slice and whole tensors stream as one large DMA (small
    # per-tile DMAs are dispatch-bound at ~113GB/s per queue).
    xP = din("xP", [128, HCH * S])
    wqP = din("wqP", [128, HCH * DL])
    wkP = din("wkP", [128, HCH * DL])
    wvP = din("wvP", [128, HCH * DL])
    woP = din("woP", [128, LH * HID])
    bq2 = din("bq2", [128, LH], f32)
    bk2 = din("bk2", [128, LH], f32)
    cosT = din("cosT", [128, S])
    sinT = din("sinT", [128, S])
    trim = din("trim", [128, 128])
    onec32 = din("onec32", [128, 32])
    j32m = din("j32", [128, 128])
    out = nc.dram_tensor("out", [S, HID], bf16, kind="ExternalOutput").ap()

    with tile.TileContext(nc) as tc, ExitStack() as ctx:
        P = ctx.enter_context(tc.tile_pool(name="persist", bufs=1))
        WQP = ctx.enter_context(tc.tile_pool(name="wq", bufs=1))
        WOP = ctx.enter_context(tc.tile_pool(name="wo", bufs=1))

        K_sb = [P.tile([128, S], bf16, tag=f"K{d}", name=f"Ksb{d}")
                for d in range(LH)]
        Q_sb = [P.tile([128, S], bf16, tag=f"Q{d}", name=f"Qsb{d}")
                for d in range(LH)]
        V_sb = [P.tile([128, DL], bf16, tag=f"V{t}", name=f"Vsb{t}")
                for t in range(S // 128)]
        cos_sb = P.tile([128, S], bf16, tag="cos")
        sin_sb = P.tile([128, S], bf16, tag="sin")
        tri_sb = P.tile([128, 128], bf16, tag="tri")
        bq_sb = P.tile([128, LH], f32, tag="bq")
        bk_sb = P.tile([128, LH], f32, tag="bk")
        onec32_sb = P.tile([128, 32], bf16, tag="onec32")
        j32_sb = P.tile([128, 128], bf16, tag="j32")
        wo_big = WOP.tile([128, LH * HID], bf16, tag="wo", name="wosb")
        wo_sb = [wo_big[:, HID * h:HID * (h + 1)] for h in range(LH)]

        def rope(pool, raw, dst, sl):
            """dst = raw*cos + rotate_half(raw)*sin; the rotate is folded
            into partition-offset reads against a half-sign-flipped sin
            table (sin_sb rows >=64 carry the minus sign)."""
            m1 = pool.tile([128, SC], bf16, tag="rm1", bufs=2)
            nc.vector.tensor_mul(m1[:], raw[:], cos_sb[:, sl])
            m2 = pool.tile([128, SC], bf16, tag="rm2", bufs=2)
            nc.vector.tensor_mul(m2[0:64, :], raw[64:128, :],
                                 sin_sb[64:128, sl])
            nc.vector.tensor_mul(m2[64:128, :], raw[0:64, :],
                                 sin_sb[0:64, sl])
            nc.vector.tensor_add(dst, m1[:], m2[:])

        # ---- phase 1: K^T (roped), V, Q^T (roped) for the whole sequence ----
        with tc.tile_pool(name="p1w", bufs=1) as WP, \
             tc.tile_pool(name="p1x", bufs=1) as XP, \
             tc.tile_pool(name="p1t", bufs=2) as TP, \
             tc.tile_pool(name="p1pa", bufs=4, space="PSUM") as PA, \
             tc.tile_pool(name="p1pb", bufs=4, space="PSUM") as PB:
            wk_big = WP.tile([128, HCH * DL], bf16, tag="wk", name="wksb")
            wv_big = WP.tile([128, HCH * DL], bf16, tag="wv", name="wvsb")
            wq_big = WQP.tile([128, HCH * DL], bf16, tag="wq", name="wqsb")
            wk_sb = [wk_big[:, DL * h:DL * (h + 1)] for h in range(HCH)]
            wv_sb = [wv_big[:, DL * h:DL * (h + 1)] for h in range(HCH)]
            wq_sb = [wq_big[:, DL * h:DL * (h + 1)] for h in range(HCH)]
            # one chunk of x tiles in flight plus the next being fetched
            x_big = [XP.tile([128, HCH * SC], bf16, tag=f"xb{par}",
                             name=f"xbig{par}") for par in range(2)]
            x_t = [[x_big[j % 2][:, SC * h:SC * (h + 1)]
                    for h in range(HCH)] for j in range(NSC)]

            # startup: the 32 wk/x0 tiles stream one 128KB tile per DMA,
            # round-robined across all three DMA queues in consumption
            # order.  Tile granularity keeps every PE wait well under the
            # HAM window (a single gap >3.4us re-throttles the PE to half
            # clock), and 3 queues x ~1.13us/tile sustain a tile pair per
            # ~0.75us against chunk-0 K's 0.86us consumption.
            qs = [nc.sync, nc.scalar, nc.gpsimd]
            for h in range(HCH):
                qs[(2 * h) % 3].dma_start(wk_big[:, DL * h:DL * (h + 1)],
                                          wkP[:, DL * h:DL * (h + 1)])
                qs[(2 * h + 1) % 3].dma_start(
                    x_big[0][:, SC * h:SC * (h + 1)],
                    xP[:, SC * h:SC * (h + 1)])
            nc.scalar.dma_start(wv_big[:], wvP[:])
            nc.gpsimd.dma_start(bq_sb[:], bq2[:])
            nc.gpsimd.dma_start(bk_sb[:], bk2[:])
            nc.gpsimd.dma_start(cos_sb[:], cosT[:])
            nc.gpsimd.dma_start(sin_sb[:], sinT[:])
            nc.gpsimd.dma_start(tri_sb[:], trim[:])
            nc.gpsimd.dma_start(onec32_sb[:], onec32[:])
            nc.gpsimd.dma_start(j32_sb[:], j32m[:])
            # trigger the one-off ACT_TABLE_LOAD while the PE is projecting
            warm = TP.tile([128, 1], f32, tag="warm", bufs=1)
            nc.scalar.activation(warm[:], bk_sb[:, 0:1], Exp)
            # PE clock warmup: the PE runs at half clock until it has been
            # busy ~3us, and it idles ~4us waiting for the first wk/x0 DMA
            # pieces - burn that wait on scratch matmuls so the real
            # projections start at full speed.
            wsc = TP.tile([128, SC], bf16, tag="wsc", bufs=1)
            nc.vector.memset(wsc[:], 0.0)
            wps = PA.tile([128, SC], f32, tag="a", name="warmps")
            for i in range(12):
                nc.tensor.matmul(wps[:], wsc[:, 0:128], wsc[:],
                                 start=True, stop=True)
            nc.sync.dma_start(wq_big[:], wqP[:])
            nc.sync.dma_start(x_big[1][:], xP[:, HCH * SC:2 * HCH * SC])
            nc.sync.dma_start(wo_big[:], woP[:])

            for j in range(NSC):
                sl = slice(SC * j, SC * (j + 1))
                if j >= 1:
                    nj = j + 1
                    if nj < NSC:
                        nc.sync.dma_start(
                            x_big[nj % 2][:],
                            xP[:, HCH * SC * nj:HCH * SC * (nj + 1)])
                # PSUM ping-pong across chunks: K/Q on pool A and V on pool
                # B for even chunks, swapped for odd chunks.  Every segment
                # then starts on banks whose evictions finished at least
                # half a segment earlier, so no segment-start stalls.
                # parity chosen so the LAST chunk's V lands on pool B
                # (banks 4-7): those drain early (V copies run during Q)
                # and phase 2's first-touched score pool PS is created
                # third, landing exactly on banks 4-7.
                KQP, VP = (PA, PB) if j % 2 == 1 else (PB, PA)
                tkq = "a" if j % 2 == 1 else "b"
                tv = "b" if j % 2 == 1 else "a"
                psk = [KQP.tile([128, SC], f32, tag=tkq,
                                name=f"psk{j}_{d}") for d in range(LH)]
                for h in range(HCH):
                    for d in range(LH):
                        nc.tensor.matmul(psk[d][:],
                                         wk_sb[h][:, 128 * d:128 * (d + 1)],
                                         x_t[j][h][:], start=(h == 0),
                                         stop=(h == HCH - 1))
                psv = [VP.tile([128, DL], f32, tag=tv,
                               name=f"psv{j}_{st}") for st in range(4)]
                for h in range(HCH):
                    for st in range(4):
                        nc.tensor.matmul(psv[st][:],
                                         x_t[j][h][:, 128 * st:128 * (st + 1)],
                                         wv_sb[h][:], start=(h == 0),
                                         stop=(h == HCH - 1))
                for d in range(LH):
                    kraw = TP.tile([128, SC], bf16, tag="kraw", bufs=2)
                    nc.scalar.activation(kraw[:], psk[d][:], Ident,
                                         bias=bk_sb[:, d:d + 1], scale=1.0)
                    rope(TP, kraw, K_sb[d][:, sl], sl)
                psq = [KQP.tile([128, SC], f32, tag=tkq,
                                name=f"psq{j}_{d}") for d in range(LH)]
                for h in range(HCH):
                    for d in range(LH):
                        nc.tensor.matmul(psq[d][:],
                                         wq_sb[h][:, 128 * d:128 * (d + 1)],
                                         x_t[j][h][:], start=(h == 0),
                                         stop=(h == HCH - 1))
                for st in range(4):
                    nc.scalar.copy(V_sb[4 * j + st][:], psv[st][:])
                for d in range(LH):
                    qraw = TP.tile([128, SC], bf16, tag="qraw", bufs=2)
                    nc.scalar.activation(qraw[:], psq[d][:], Ident,
                                         bias=bq_sb[:, d:d + 1], scale=1.0)
                    rope(TP, qraw, Q_sb[d][:, sl], sl)

        # ---- phase 2: attention (pipelined) + deferred out-projection ----
        # pool creation order fixes bank placement: SH+PC on the psq banks
        # (drained right after phase 1), PS on the psv banks (drained early).
        with tc.tile_pool(name="p2sh", bufs=2, space="PSUM") as SH, \
             tc.tile_pool(name="p2pc", bufs=2, space="PSUM") as PC, \
             tc.tile_pool(name="p2ps", bufs=4, space="PSUM") as PS, \
             tc.tile_pool(name="p2ex", bufs=8) as EX, \
             tc.tile_pool(name="p2e4", bufs=2) as E4P, \
             tc.tile_pool(name="p2rb", bufs=2) as RB, \
             tc.tile_pool(name="p2ct", bufs=2) as CT, \
             tc.tile_pool(name="p2ot", bufs=4) as OT:

            def outproj_chain(j, ct, oc, qt, last=False):
                """One 4-matmul out-projection chain; the copy rides on DVE
                (Act is exp-saturated while these interleave attention)."""
                osl = slice(SC * oc, SC * (oc + 1))
                pso = PS.tile([128, SC], f32, tag="ps",
                              name=f"pso{j}_{oc}_{qt}")
                for it in range(LH):
                    nc.tensor.matmul(
                        pso[:], ct[it][:, 128 * qt:128 * (qt + 1)],
                        wo_sb[it][:, osl], start=(it == 0),
                        stop=(it == LH - 1))
                ot = OT.tile([128, SC], bf16, tag="ot",
                             name=f"ot{j}_{oc}_{qt}")
                if last and qt % 2 == 0:
                    nc.scalar.copy(ot[:], pso[:])
                else:
                    nc.vector.tensor_copy(ot[:], pso[:])
                # the final chunk's stores split across both HWDGE
                # queues so the drain after the last matmul halves
                dma_eng = nc.scalar if (last and qt % 2) else nc.sync
                dma_eng.dma_start(
                    out[SC * j + 128 * qt:SC * j + 128 * (qt + 1),
                        osl], ot[:])

            def emit_outproj(j, ct, last=False):
                for oc in range(4):
                    for qt in range(4):
                        outproj_chain(j, ct, oc, qt, last)

            pending = None   # (j, ct) outproj deferred one chunk
            ct = [None] * LH

            def emit_norm_tail(j, h, psc, psd4):
                """denominator merge (cast + 1/32-ones matmul summing the 4
                col-tiled partial chains and broadcasting across partitions)
                + reciprocal + ct mul for head h of chunk j (deferred one
                head so the PE never waits on the cast)."""
                rows = 32 if j == 0 else 96
                e4 = E4P.tile([128, SC], bf16, tag="e4", name=f"e4_{j}_{h}")
                nc.vector.tensor_copy(e4[0:rows, :], psd4[0:rows, :])
                psb = SH.tile([128, SC], f32, tag="sh", name=f"psb{j}_{h}")
                nc.tensor.matmul(psb[:], j32_sb[0:rows, :], e4[0:rows, :],
                                 start=True, stop=True)
                rb = RB.tile([128, SC], f32, tag="rb", name=f"rb{j}_{h}")
                nc.vector.reciprocal_approx_fast(out=rb[:], in_=psb[:])
                cth = CT.tile([128, SC], bf16, tag=f"ct{h}",
                              name=f"ct{j}_{h}")
                nc.vector.tensor_mul(cth[:], psc[:], rb[:])
                ct[h] = cth

            for j in range(NSC):
                T = 4 * j + 4
                norm_pend = None
                # interleave the previous chunk's out-projection chains into
                # this chunk's attention tile stream: the PE fills the slack
                # of the (slightly slower) exp pipeline instead of running
                # the out-projection as an Act-idle block afterwards.
                op_j, op_ct = pending if pending is not None else (None, None)
                op_emitted, tiles_done, tiles_total = 0, 0, LH * T

                for h in range(LH):
                    psc = PC.tile([128, SC], f32, tag="pc",
                                  name=f"psc{j}_{h}")
                    psd4 = SH.tile([128, SC], f32, tag="sh",
                                   name=f"psd4_{j}_{h}")
                    exs = [None] * T

                    def emit_score(t):
                        p = t - 4 * j  # >=0 for diagonal tiles
                        c0 = 128 * p if p > 0 else 0
                        cs = slice(c0, SC)
                        ps = PS.tile([128, SC], f32, tag="ps",
                                     name=f"pss{j}_{h}_{t}")
                        nc.tensor.matmul(ps[:, cs],
                                         K_sb[h][:, 128 * t:128 * (t + 1)],
                                         Q_sb[h][:, SC * j + c0:SC * (j + 1)],
                                         start=True, stop=True)
                        ex = EX.tile([128, SC], bf16, tag="ex",
                                     name=f"ex{j}_{h}_{t}")
                        nc.scalar.activation(ex[:, cs], ps[:, cs], Exp)
                        if p >= 0:
                            # mask on the idle gpsimd engine: on DVE these
                            # muls (which wait on exp) delay the pso copies
                            # queued behind them, holding PS banks
                            dsl = slice(128 * p, 128 * (p + 1))
                            nc.gpsimd.tensor_mul(ex[:, dsl], ex[:, dsl],
                                                 tri_sb[:])
                        exs[t] = ex

                    def emit_psd(t):
                        # denominator partial: chain t%3 col-tiled at
                        # partitions 32*(t%3) (AP base partition supports
                        # only 0/32/64), M=32 copies via the ones lhsT;
                        # groups of 3 consecutive emissions run
                        # concurrently on disjoint PE column groups.
                        p = t - 4 * j
                        c0 = 128 * p if p > 0 else 0
                        cs = slice(c0, SC)
                        if j == 0:
                            # single sequential chain (first tile of the
                            # other chains would start on a partial width)
                            nc.tensor.matmul(psd4[0:32, cs], onec32_sb[:],
                                             exs[t][:, cs],
                                             start=(t == 0),
                                             stop=(t == T - 1))
                        else:
                            g = t % 3
                            nc.tensor.matmul(psd4[32 * g:32 * (g + 1), cs],
                                             onec32_sb[:], exs[t][:, cs],
                                             start=(t < 3),
                                             stop=(t >= T - 3))

                    for t in range(min(3, T)):
                        emit_score(t)
                    # two chains right after the prologue fill the PE while
                    # the head's exp pipeline warms up (psc_0 would
                    # otherwise stall ~0.5us on the first exps)
                    for _ in range(2):
                        if op_j is not None and op_emitted < 16:
                            outproj_chain(op_j, op_ct, op_emitted // 4,
                                          op_emitted % 4)
                            op_emitted += 1
                    # previous head's norm tail goes here — before this
                    # head's first psd4 group, which reuses the SH bank the
                    # tail's cast reads (emitting it later would deadlock
                    # the Tensor queue against the DVE queue)
                    if norm_pend is not None:
                        emit_norm_tail(*norm_pend)
                        norm_pend = None
                    for t in range(T):
                        if t + 3 < T:
                            emit_score(t + 3)
                        p = t - 4 * j
                        cs = slice(128 * p if p > 0 else 0, SC)
                        nc.tensor.matmul(psc[:, cs],
                                         V_sb[t][:, 128 * h:128 * (h + 1)],
                                         exs[t][:, cs], start=(t == 0),
                                         stop=(t == T - 1))
                        if j == 0:
                            emit_psd(t)
                        elif t % 3 == 2 or t == T - 1:
                            lo = t - t % 3 if t % 3 == 2 else T - T % 3
                            for tt in range(lo, t + 1):
                                emit_psd(tt)
                        tiles_done += 1
                        while (op_j is not None and op_emitted < 16 and
                               16 * tiles_done >= tiles_total *
                               (op_emitted + 1)):
                            outproj_chain(op_j, op_ct, op_emitted // 4,
                                          op_emitted % 4)
                            op_emitted += 1
                    norm_pend = (j, h, psc, psd4)
                while op_j is not None and op_emitted < 16:
                    outproj_chain(op_j, op_ct, op_emitted // 4,
                                  op_emitted % 4)
                    op_emitted += 1
                emit_norm_tail(*norm_pend)
                pending = (j, list(ct))
            emit_outproj(*pending, last=True)
    nc.compile()
    return nc


def _get_nc():
    if "nc" not in _CACHE:
        _CACHE["nc"] = _build_nc()
    return _CACHE["nc"]


def _consts():
    if "consts" not in _CACHE:
        inv = (10000.0 ** (-np.arange(0, D, 2, dtype=np.float64) / D))
        t = np.arange(S, dtype=np.float64)
        fr = np.outer(t, inv)                      # [S, 64]
        cos = np.concatenate([np.cos(fr)] * 2, 1).T
        sin = np.concatenate([np.sin(fr)] * 2, 1).T.copy()
        sin[64:] *= -1.0
        tri = (np.arange(128)[:, None] <= np.arange(128)[None, :])
        _CACHE["consts"] = {
            "cosT": np.ascontiguousarray(cos.astype(ml_dtypes.bfloat16)),
            "sinT": np.ascontiguousarray(sin.astype(ml_dtypes.bfloat16)),
            "trim": np.ascontiguousarray(
                tri.astype(ml_dtypes.bfloat16)),
            "onec32": np.ones((128, 32), ml_dtypes.bfloat16),
            "j32": np.full((128, 128), 1.0 / 32, ml_dtypes.bfloat16),
        }
    return _CACHE["consts"]


def _pmaj(a):
    """[128*n, W] -> partition-major [128, n*W] (block b at free offset
    b*W on every partition)."""
    n = a.shape[0] // 128
    return np.ascontiguousarray(
        a.reshape(n, 128, a.shape[1]).transpose(1, 0, 2).reshape(128, -1))


def _marshal(hidden_states, Wq, bq, Wk, bk, Wv, bv, Wo, bo):
    consts = _consts()
    scale = 1.0 / np.sqrt(D)
    xPs = []
    for b in range(2):
        xT = hidden_states[b].T.astype(ml_dtypes.bfloat16)  # [HID, S]
        # xP[p, S*h/?]: chunk-major: [p, HCH*SC*j + SC*h + c]
        xPs.append(np.ascontiguousarray(
            xT.reshape(HCH, 128, NSC, SC).transpose(1, 2, 0, 3)
            .reshape(128, HCH * S)))
    in_maps = []
    for c in range(N_CORES):
        b, hg = c // 4, c % 4
        rows = slice(DL * hg, DL * (hg + 1))
        m = dict(consts)
        m["xP"] = xPs[b]
        m["wqP"] = _pmaj((Wq[rows] * scale).T.astype(ml_dtypes.bfloat16))
        m["wkP"] = _pmaj(Wk[rows].T.astype(ml_dtypes.bfloat16))
        m["wvP"] = _pmaj(Wv[rows].T.astype(ml_dtypes.bfloat16))
        m["woP"] = _pmaj(Wo[:, rows].T.astype(ml_dtypes.bfloat16))
        m["bq2"] = np.ascontiguousarray(
            (bq[rows] * scale).reshape(LH, 128).T.astype(np.float32))
        m["bk2"] = np.ascontiguousarray(
            bk[rows].reshape(LH, 128).T.astype(np.float32))
        in_maps.append(m)
    return in_maps


def _gather(results, bias):
    out = np.empty((2, S, HID), np.float32)
    for b in range(2):
        acc = results[4 * b]["out"].astype(np.float32).copy()
        for g in range(1, 4):
            acc += results[4 * b + g]["out"]
        out[b] = acc + bias
    return out


def _run(inputs, **kw):
    nc = _get_nc()
    in_maps = _marshal(**{k: np.asarray(v) for k, v in inputs.items()})
    return run_bass_kernel_spmd(nc, in_maps, core_ids=list(range(N_CORES)),
                                **kw)


def _host_bias(inputs):
    Wo = np.asarray(inputs["Wo"], np.float64)
    bv = np.asarray(inputs["bv"], np.float64)
    bo = np.asarray(inputs["bo"], np.float64)
    return (bo + Wo @ bv).astype(np.float32)


def kernel(**inputs):
    res = _run(inputs)
    return _gather(res.results, _host_bias(inputs))


def kernel_traced(**inputs):
    """Like kernel() but with NTFF profiling; returns (output, results)."""
    import types

    try:
        import antenv.axon_hooks  # noqa: F401
    except ImportError:
        from trn_agent_boot.trn_boot import _ntff_profile_via_ctypes
        hook = _ntff_profile_via_ctypes("/opt/axon/libaxon_pjrt.so")
        mod = types.ModuleType("antenv.axon_hooks")
        mod.get_axon_ntff_profile_hook = lambda: hook
        mod.set_axon_ntff_profile_hook = lambda h: None
        sys.modules["antenv.axon_hooks"] = mod
    res = _run(inputs, trace=True)
    return _gather(res.results, _host_bias(inputs)), res



# revision 38
# speedup vs baseline: 1.0123x; 1.0123x over previous
"""Dense causal MHA (B=2, S=2048, H=16, D=128, hidden=2048) on 8 Trainium2 cores.

Sharding: data-parallel over batch (2) x tensor-parallel over head groups
(4 heads/core).  Core c handles batch c//4, heads 4*(c%4) .. 4*(c%4)+3.
Each core computes a partial output (its heads' contribution to the out
projection, with bo/4 folded in); the host sums the 4 partials per batch.

v4 layout (everything bf16 except PSUM accumulation, reciprocals and the
final output, which stay f32):
  - single pass over x: phase 1 computes K^T (roped), V and Q^T (roped) for
    the whole sequence; K/Q and V ping-pong between the two 4-bank PSUM
    pools across chunks so no segment waits on Act evictions.
  - weights and x live in DRAM in partition-major layout ([128, blocks*W])
    so whole tensors stream as a few large DMAs (small per-tile DMAs are
    dispatch-bound at ~113GB/s/queue); the startup-critical wk and x0 go
    in progressively larger pieces on two dedicated queues.
  - phase 2 is attention + out-projection only, software-pipelined with a
    3-tile lookahead so the PE never waits on the exp chain.
  - softmax denominator: three M=32 ones-matmul chains col-tiled at
    partitions 0/32/64 of one bank (groups of 3 consecutive emissions run
    concurrently on disjoint PE column groups, ~2x cheaper than M=1
    chains); a [128,128] (1/32)-matmul then sums the chain partials and
    broadcasts the denominator across partitions in one shot, deferred one
    head so the PE never waits on the DVE cast feeding it.
  - out-projection deferred one chunk so its matmuls fill the PE while the
    next chunk's attention warms up; the last chunk's stores split across
    both HWDGE queues.
"""

import sys

sys.path.insert(0, "/opt/trn_rl_repo")

from contextlib import ExitStack

import numpy as np
import ml_dtypes

import concourse.tile as tile
from concourse import bacc, mybir
from concourse.bass_utils import run_bass_kernel_spmd

S = 2048
HID = 2048
D = 128
LH = 4            # heads per core
DL = LH * D       # 512 local inner dims
SC = 512          # chunk size (q and kv)
NSC = S // SC     # 4
HCH = HID // 128  # 16 contraction chunks
N_CORES = 8

f32 = mybir.dt.float32
f32r = mybir.dt.float32r
bf16 = mybir.dt.bfloat16
Exp = mybir.ActivationFunctionType.Exp
Ident = mybir.ActivationFunctionType.Identity

_CACHE = {}


def _build_nc():
    nc = bacc.Bacc("TRN2", target_bir_lowering=False, debug=False,
                   num_devices=N_CORES)

    def din(name, shape, dt=bf16):
        return nc.dram_tensor(name, shape, dt, kind="ExternalInput").ap()

    # partition-major layouts: element [p, blk*W + c] holds the logical
    # [128*blk + p, c] entry, so each 128-row tile block is a contiguous
    # free-dim slice and whole tensors stream as one large DMA (small
    # per-tile DMAs are dispatch-bound at ~113GB/s per queue).
    xP = din("xP", [128, HCH * S])
    wqP = din("wqP", [128, HCH * DL])
    wkP = din("wkP", [128, HCH * DL])
    wvP = din("wvP", [128, HCH * DL])
    woP = din("woP", [128, LH * HID])
    bq2 = din("bq2", [128, LH], f32)
    bk2 = din("bk2", [128, LH], f32)
    cosT = din("cosT", [128, S])
    sinT = din("sinT", [128, S])
    trim = din("trim", [128, 128])
    onec32 = din("onec32", [128, 32])
    j32m = din("j32", [128, 128])
    out = nc.dram_tensor("out", [S, HID], bf16, kind="ExternalOutput").ap()

    with tile.TileContext(nc) as tc, ExitStack() as ctx:
        P = ctx.enter_context(tc.tile_pool(name="persist", bufs=1))
        WQP = ctx.enter_context(tc.tile_pool(name="wq", bufs=1))
        WOP = ctx.enter_context(tc.tile_pool(name="wo", bufs=1))

        K_sb = [P.tile([128, S], bf16, tag=f"K{d}", name=f"Ksb{d}")
                for d in range(LH)]
        Q_sb = [P.tile([128, S], bf16, tag=f"Q{d}", name=f"Qsb{d}")
                for d in range(LH)]
        V_sb = [P.tile([128, DL], bf16, tag=f"V{t}", name=f"Vsb{t}")
                for t in range(S // 128)]
        cos_sb = P.tile([128, S], bf16, tag="cos")
        sin_sb = P.tile([128, S], bf16, tag="sin")
        tri_sb = P.tile([128, 128], bf16, tag="tri")
        bq_sb = P.tile([128, LH], f32, tag="bq")
        bk_sb = P.tile([128, LH], f32, tag="bk")
        onec32_sb = P.tile([128, 32], bf16, tag="onec32")
        j32_sb = P.tile([128, 128], bf16, tag="j32")
        wo_big = WOP.tile([128, LH * HID], bf16, tag="wo", name="wosb")
        wo_sb = [wo_big[:, HID * h:HID * (h + 1)] for h in range(LH)]

        def rope(pool, raw, dst, sl):
            """dst = raw*cos + rotate_half(raw)*sin; the rotate is folded
            into partition-offset reads against a half-sign-flipped sin
            table (sin_sb rows >=64 carry the minus sign)."""
            m1 = pool.tile([128, SC], bf16, tag="rm1", bufs=2)
            nc.vector.tensor_mul(m1[:], raw[:], cos_sb[:, sl])
            m2 = pool.tile([128, SC], bf16, tag="rm2", bufs=2)
            nc.vector.tensor_mul(m2[0:64, :], raw[64:128, :],
                                 sin_sb[64:128, sl])
            nc.vector.tensor_mul(m2[64:128, :], raw[0:64, :],
                                 sin_sb[0:64, sl])
            nc.vector.tensor_add(dst, m1[:], m2[:])

        # ---- phase 1: K^T (roped), V, Q^T (roped) for the whole sequence ----
        with tc.tile_pool(name="p1w", bufs=1) as WP, \
             tc.tile_pool(name="p1x", bufs=1) as XP, \
             tc.tile_pool(name="p1t", bufs=2) as TP, \
             tc.tile_pool(name="p1pa", bufs=4, space="PSUM") as PA, \
             tc.tile_pool(name="p1pb", bufs=4, space="PSUM") as PB:
            wk_big = WP.tile([128, HCH * DL], bf16, tag="wk", name="wksb")
            wv_big = WP.tile([128, HCH * DL], bf16, tag="wv", name="wvsb")
            wq_big = WQP.tile([128, HCH * DL], bf16, tag="wq", name="wqsb")
            wk_sb = [wk_big[:, DL * h:DL * (h + 1)] for h in range(HCH)]
            wv_sb = [wv_big[:, DL * h:DL * (h + 1)] for h in range(HCH)]
            wq_sb = [wq_big[:, DL * h:DL * (h + 1)] for h in range(HCH)]
            # one chunk of x tiles in flight plus the next being fetched
            x_big = [XP.tile([128, HCH * SC], bf16, tag=f"xb{par}",
                             name=f"xbig{par}") for par in range(2)]
            x_t = [[x_big[j % 2][:, SC * h:SC * (h + 1)]
                    for h in range(HCH)] for j in range(NSC)]

            # startup: wk and x0 stream one 128KB tile per DMA on three
            # queues (wk on SP, x0 even tiles on Act, odd on Pool).  Tile
            # granularity keeps PE waits short (a single gap >3.4us makes
            # the HAM re-throttle the PE to half clock); round-robining
            # BOTH streams over the queues measured worse (the slower
            # Pool-queue dispatch head-of-line blocks both streams).
            for h in range(HCH):
                nc.sync.dma_start(wk_big[:, DL * h:DL * (h + 1)],
                                  wkP[:, DL * h:DL * (h + 1)])
                xq = nc.scalar if h % 2 == 0 else nc.gpsimd
                xq.dma_start(x_big[0][:, SC * h:SC * (h + 1)],
                             xP[:, SC * h:SC * (h + 1)])
            nc.scalar.dma_start(wv_big[:], wvP[:])
            nc.gpsimd.dma_start(bq_sb[:], bq2[:])
            nc.gpsimd.dma_start(bk_sb[:], bk2[:])
            nc.gpsimd.dma_start(cos_sb[:], cosT[:])
            nc.gpsimd.dma_start(sin_sb[:], sinT[:])
            nc.gpsimd.dma_start(tri_sb[:], trim[:])
            nc.gpsimd.dma_start(onec32_sb[:], onec32[:])
            nc.gpsimd.dma_start(j32_sb[:], j32m[:])
            # trigger the one-off ACT_TABLE_LOAD while the PE is projecting
            warm = TP.tile([128, 1], f32, tag="warm", bufs=1)
            nc.scalar.activation(warm[:], bk_sb[:, 0:1], Exp)
            # PE clock warmup: the PE runs at half clock until it has been
            # busy ~3us, and it idles ~4us waiting for the first wk/x0 DMA
            # pieces - burn that wait on scratch matmuls so the real
            # projections start at full speed.
            wsc = TP.tile([128, SC], bf16, tag="wsc", bufs=1)
            nc.vector.memset(wsc[:], 0.0)
            wps = PA.tile([128, SC], f32, tag="a", name="warmps")
            for i in range(12):
                nc.tensor.matmul(wps[:], wsc[:, 0:128], wsc[:],
                                 start=True, stop=True)
            nc.sync.dma_start(wq_big[:], wqP[:])
            nc.sync.dma_start(x_big[1][:], xP[:, HCH * SC:2 * HCH * SC])
            nc.sync.dma_start(wo_big[:], woP[:])

            for j in range(NSC):
                sl = slice(SC * j, SC * (j + 1))
                if j >= 1:
                    nj = j + 1
                    if nj < NSC:
                        nc.sync.dma_start(
                            x_big[nj % 2][:],
                            xP[:, HCH * SC * nj:HCH * SC * (nj + 1)])
                # PSUM ping-pong across chunks: K/Q on pool A and V on pool
                # B for even chunks, swapped for odd chunks.  Every segment
                # then starts on banks whose evictions finished at least
                # half a segment earlier, so no segment-start stalls.
                # parity chosen so the LAST chunk's V lands on pool B
                # (banks 4-7): those drain early (V copies run during Q)
                # and phase 2's first-touched score pool PS is created
                # third, landing exactly on banks 4-7.
                KQP, VP = (PA, PB) if j % 2 == 1 else (PB, PA)
                tkq = "a" if j % 2 == 1 else "b"
                tv = "b" if j % 2 == 1 else "a"
                psk = [KQP.tile([128, SC], f32, tag=tkq,
                                name=f"psk{j}_{d}") for d in range(LH)]
                for h in range(HCH):
                    for d in range(LH):
                        nc.tensor.matmul(psk[d][:],
                                         wk_sb[h][:, 128 * d:128 * (d + 1)],
                                         x_t[j][h][:], start=(h == 0),
                                         stop=(h == HCH - 1))
                psv = [VP.tile([128, DL], f32, tag=tv,
                               name=f"psv{j}_{st}") for st in range(4)]
                for h in range(HCH):
                    for st in range(4):
                        nc.tensor.matmul(psv[st][:],
                                         x_t[j][h][:, 128 * st:128 * (st + 1)],
                                         wv_sb[h][:], start=(h == 0),
                                         stop=(h == HCH - 1))
                for d in range(LH):
                    kraw = TP.tile([128, SC], bf16, tag="kraw", bufs=2)
                    nc.scalar.activation(kraw[:], psk[d][:], Ident,
                                         bias=bk_sb[:, d:d + 1], scale=1.0)
                    rope(TP, kraw, K_sb[d][:, sl], sl)
                psq = [KQP.tile([128, SC], f32, tag=tkq,
                                name=f"psq{j}_{d}") for d in range(LH)]
                for h in range(HCH):
                    for d in range(LH):
                        nc.tensor.matmul(psq[d][:],
                                         wq_sb[h][:, 128 * d:128 * (d + 1)],
                                         x_t[j][h][:], start=(h == 0),
                                         stop=(h == HCH - 1))
                for st in range(4):
                    nc.scalar.copy(V_sb[4 * j + st][:], psv[st][:])
                for d in range(LH):
                    qraw = TP.tile([128, SC], bf16, tag="qraw", bufs=2)
                    nc.scalar.activation(qraw[:], psq[d][:], Ident,
                                         bias=bq_sb[:, d:d + 1], scale=1.0)
                    rope(TP, qraw, Q_sb[d][:, sl], sl)

        # ---- phase 2: attention (pipelined) + deferred out-projection ----
        # pool creation order fixes bank placement: SH+PC on the psq banks
        # (drained right after phase 1), PS on the psv banks (drained early).
        with tc.tile_pool(name="p2sh", bufs=2, space="PSUM") as SH, \
             tc.tile_pool(name="p2pc", bufs=2, space="PSUM") as PC, \
             tc.tile_pool(name="p2ps", bufs=4, space="PSUM") as PS, \
             tc.tile_pool(name="p2ex", bufs=8) as EX, \
             tc.tile_pool(name="p2e4", bufs=2) as E4P, \
             tc.tile_pool(name="p2rb", bufs=2) as RB, \
             tc.tile_pool(name="p2ct", bufs=2) as CT, \
             tc.tile_pool(name="p2ot", bufs=4) as OT:

            def outproj_chain(j, ct, oc, qt, last=False):
                """One 4-matmul out-projection chain; the copy rides on DVE
                (Act is exp-saturated while these interleave attention)."""
                osl = slice(SC * oc, SC * (oc + 1))
                pso = PS.tile([128, SC], f32, tag="ps",
                              name=f"pso{j}_{oc}_{qt}")
                for it in range(LH):
                    nc.tensor.matmul(
                        pso[:], ct[it][:, 128 * qt:128 * (qt + 1)],
                        wo_sb[it][:, osl], start=(it == 0),
                        stop=(it == LH - 1))
                ot = OT.tile([128, SC], bf16, tag="ot",
                             name=f"ot{j}_{oc}_{qt}")
                if last and qt % 2 == 0:
                    nc.scalar.copy(ot[:], pso[:])
                else:
                    nc.vector.tensor_copy(ot[:], pso[:])
                # the final chunk's stores split across both HWDGE
                # queues so the drain after the last matmul halves
                dma_eng = nc.scalar if (last and qt % 2) else nc.sync
                dma_eng.dma_start(
                    out[SC * j + 128 * qt:SC * j + 128 * (qt + 1),
                        osl], ot[:])

            def emit_outproj(j, ct, last=False):
                for oc in range(4):
                    for qt in range(4):
                        outproj_chain(j, ct, oc, qt, last)

            pending = None   # (j, ct) outproj deferred one chunk
            ct = [None] * LH

            def emit_norm_tail(j, h, psc, psd4):
                """denominator merge (cast + 1/32-ones matmul summing the 4
                col-tiled partial chains and broadcasting across partitions)
                + reciprocal + ct mul for head h of chunk j (deferred one
                head so the PE never waits on the cast)."""
                rows = 32 if j == 0 else 96
                e4 = E4P.tile([128, SC], bf16, tag="e4", name=f"e4_{j}_{h}")
                nc.vector.tensor_copy(e4[0:rows, :], psd4[0:rows, :])
                psb = SH.tile([128, SC], f32, tag="sh", name=f"psb{j}_{h}")
                nc.tensor.matmul(psb[:], j32_sb[0:rows, :], e4[0:rows, :],
                                 start=True, stop=True)
                rb = RB.tile([128, SC], f32, tag="rb", name=f"rb{j}_{h}")
                nc.vector.reciprocal_approx_fast(out=rb[:], in_=psb[:])
                cth = CT.tile([128, SC], bf16, tag=f"ct{h}",
                              name=f"ct{j}_{h}")
                nc.vector.tensor_mul(cth[:], psc[:], rb[:])
                ct[h] = cth

            for j in range(NSC):
                T = 4 * j + 4
                norm_pend = None
                # interleave the previous chunk's out-projection chains into
                # this chunk's attention tile stream: the PE fills the slack
                # of the (slightly slower) exp pipeline instead of running
                # the out-projection as an Act-idle block afterwards.
                op_j, op_ct = pending if pending is not None else (None, None)
                op_emitted, tiles_done, tiles_total = 0, 0, LH * T

                for h in range(LH):
                    psc = PC.tile([128, SC], f32, tag="pc",
                                  name=f"psc{j}_{h}")
                    psd4 = SH.tile([128, SC], f32, tag="sh",
                                   name=f"psd4_{j}_{h}")
                    exs = [None] * T

                    def emit_score(t):
                        p = t - 4 * j  # >=0 for diagonal tiles
                        c0 = 128 * p if p > 0 else 0
                        cs = slice(c0, SC)
                        ps = PS.tile([128, SC], f32, tag="ps",
                                     name=f"pss{j}_{h}_{t}")
                        nc.tensor.matmul(ps[:, cs],
                                         K_sb[h][:, 128 * t:128 * (t + 1)],
                                         Q_sb[h][:, SC * j + c0:SC * (j + 1)],
                                         start=True, stop=True)
                        ex = EX.tile([128, SC], bf16, tag="ex",
                                     name=f"ex{j}_{h}_{t}")
                        nc.scalar.activation(ex[:, cs], ps[:, cs], Exp)
                        if p >= 0:
                            # mask on the idle gpsimd engine: on DVE these
                            # muls (which wait on exp) delay the pso copies
                            # queued behind them, holding PS banks
                            dsl = slice(128 * p, 128 * (p + 1))
                            nc.gpsimd.tensor_mul(ex[:, dsl], ex[:, dsl],
                                                 tri_sb[:])
                        exs[t] = ex

                    def emit_psd(t):
                        # denominator partial: chain t%3 col-tiled at
                        # partitions 32*(t%3) (AP base partition supports
                        # only 0/32/64), M=32 copies via the ones lhsT;
                        # groups of 3 consecutive emissions run
                        # concurrently on disjoint PE column groups.
                        p = t - 4 * j
                        c0 = 128 * p if p > 0 else 0
                        cs = slice(c0, SC)
                        if j == 0:
                            # single sequential chain (first tile of the
                            # other chains would start on a partial width)
                            nc.tensor.matmul(psd4[0:32, cs], onec32_sb[:],
                                             exs[t][:, cs],
                                             start=(t == 0),
                                             stop=(t == T - 1))
                        else:
                            g = t % 3
                            nc.tensor.matmul(psd4[32 * g:32 * (g + 1), cs],
                                             onec32_sb[:], exs[t][:, cs],
                                             start=(t < 3),
                                             stop=(t >= T - 3))

                    for t in range(min(3, T)):
                        emit_score(t)
                    # two chains right after the prologue fill the PE while
                    # the head's exp pipeline warms up (psc_0 would
                    # otherwise stall ~0.5us on the first exps)
                    for _ in range(2):
                        if op_j is not None and op_emitted < 16:
                            outproj_chain(op_j, op_ct, op_emitted // 4,
                                          op_emitted % 4)
                            op_emitted += 1
                    # previous head's norm tail goes here — before this
                    # head's first psd4 group, which reuses the SH bank the
                    # tail's cast reads (emitting it later would deadlock
                    # the Tensor queue against the DVE queue)
                    if norm_pend is not None:
                        emit_norm_tail(*norm_pend)
                        norm_pend = None
                    for t in range(T):
                        if t + 3 < T:
                            emit_score(t + 3)
                        p = t - 4 * j
                        cs = slice(128 * p if p > 0 else 0, SC)
                        nc.tensor.matmul(psc[:, cs],
                                         V_sb[t][:, 128 * h:128 * (h + 1)],
                                         exs[t][:, cs], start=(t == 0),
                                         stop=(t == T - 1))
                        if j == 0:
                            emit_psd(t)
                        elif t % 3 == 2 or t == T - 1:
                            lo = t - t % 3 if t % 3 == 2 else T - T % 3
                            for tt in range(lo, t + 1):
                                emit_psd(tt)
                        tiles_done += 1
                        while (op_j is not None and op_emitted < 16 and
                               16 * tiles_done >= tiles_total *
                               (op_emitted + 1)):
                            outproj_chain(op_j, op_ct, op_emitted // 4,
                                          op_emitted % 4)
                            op_emitted += 1
                    norm_pend = (j, h, psc, psd4)
                while op_j is not None and op_emitted < 16:
                    outproj_chain(op_j, op_ct, op_emitted // 4,
                                  op_emitted % 4)
                    op_emitted += 1
                emit_norm_tail(*norm_pend)
                pending = (j, list(ct))
            emit_outproj(*pending, last=True)
    nc.compile()
    return nc


def _get_nc():
    if "nc" not in _CACHE:
        _CACHE["nc"] = _build_nc()
    return _CACHE["nc"]


def _consts():
    if "consts" not in _CACHE:
        inv = (10000.0 ** (-np.arange(0, D, 2, dtype=np.float64) / D))
        t = np.arange(S, dtype=np.float64)
        fr = np.outer(t, inv)                      # [S, 64]
        cos = np.concatenate([np.cos(fr)] * 2, 1).T
        sin = np.concatenate([np.sin(fr)] * 2, 1).T.copy()
        sin[64:] *= -1.0
        tri = (np.arange(128)[:, None] <= np.arange(128)[None, :])
        _CACHE["consts"] = {
            "cosT": np.ascontiguousarray(cos.astype(ml_dtypes.bfloat16)),
            "sinT": np.ascontiguousarray(sin.astype(ml_dtypes.bfloat16)),
            "trim": np.ascontiguousarray(
                tri.astype(ml_dtypes.bfloat16)),
            "onec32": np.ones((128, 32), ml_dtypes.bfloat16),
            "j32": np.full((128, 128), 1.0 / 32, ml_dtypes.bfloat16),
        }
    return _CACHE["consts"]


def _pmaj(a):
    """[128*n, W] -> partition-major [128, n*W] (block b at free offset
    b*W on every partition)."""
    n = a.shape[0] // 128
    return np.ascontiguousarray(
        a.reshape(n, 128, a.shape[1]).transpose(1, 0, 2).reshape(128, -1))


def _marshal(hidden_states, Wq, bq, Wk, bk, Wv, bv, Wo, bo):
    consts = _consts()
    scale = 1.0 / np.sqrt(D)
    xPs = []
    for b in range(2):
        xT = hidden_states[b].T.astype(ml_dtypes.bfloat16)  # [HID, S]
        # xP[p, S*h/?]: chunk-major: [p, HCH*SC*j + SC*h + c]
        xPs.append(np.ascontiguousarray(
            xT.reshape(HCH, 128, NSC, SC).transpose(1, 2, 0, 3)
            .reshape(128, HCH * S)))
    in_maps = []
    for c in range(N_CORES):
        b, hg = c // 4, c % 4
        rows = slice(DL * hg, DL * (hg + 1))
        m = dict(consts)
        m["xP"] = xPs[b]
        m["wqP"] = _pmaj((Wq[rows] * scale).T.astype(ml_dtypes.bfloat16))
        m["wkP"] = _pmaj(Wk[rows].T.astype(ml_dtypes.bfloat16))
        m["wvP"] = _pmaj(Wv[rows].T.astype(ml_dtypes.bfloat16))
        m["woP"] = _pmaj(Wo[:, rows].T.astype(ml_dtypes.bfloat16))
        m["bq2"] = np.ascontiguousarray(
            (bq[rows] * scale).reshape(LH, 128).T.astype(np.float32))
        m["bk2"] = np.ascontiguousarray(
            bk[rows].reshape(LH, 128).T.astype(np.float32))
        in_maps.append(m)
    return in_maps


def _gather(results, bias):
    out = np.empty((2, S, HID), np.float32)
    for b in range(2):
        acc = results[4 * b]["out"].astype(np.float32).copy()
        for g in range(1, 4):
            acc += results[4 * b + g]["out"]
        out[b] = acc + bias
    return out


def _run(inputs, **kw):
    nc = _get_nc()
    in_maps = _marshal(**{k: np.asarray(v) for k, v in inputs.items()})
    return run_bass_kernel_spmd(nc, in_maps, core_ids=list(range(N_CORES)),
                                **kw)


def _host_bias(inputs):
    Wo = np.asarray(inputs["Wo"], np.float64)
    bv = np.asarray(inputs["bv"], np.float64)
    bo = np.asarray(inputs["bo"], np.float64)
    return (bo + Wo @ bv).astype(np.float32)


def kernel(**inputs):
    res = _run(inputs)
    return _gather(res.results, _host_bias(inputs))


def kernel_traced(**inputs):
    """Like kernel() but with NTFF profiling; returns (output, results)."""
    import types

    try:
        import antenv.axon_hooks  # noqa: F401
    except ImportError:
        from trn_agent_boot.trn_boot import _ntff_profile_via_ctypes
        hook = _ntff_profile_via_ctypes("/opt/axon/libaxon_pjrt.so")
        mod = types.ModuleType("antenv.axon_hooks")
        mod.get_axon_ntff_profile_hook = lambda: hook
        mod.set_axon_ntff_profile_hook = lambda h: None
        sys.modules["antenv.axon_hooks"] = mod
    res = _run(inputs, trace=True)
    return _gather(res.results, _host_bias(inputs)), res

